# revision 11
# baseline (speedup 1.0000x reference)
"""BlockAttention prefill kernel for Trainium2, 8-core tensor-parallel.

Reference op (see problem): scatter K/V rows into paged caches, then
block-causal (staircase, block_length=32) attention over T=4096 tokens,
16 query heads / 4 KV heads (GQA), head_dim=128, fp32.

Sharding: pure tensor parallelism over heads. Core c computes query heads
{2c, 2c+1}, which share KV head c//2. Cache update is split so core 2j
produces the K-cache slice of KV head j and core 2j+1 the V-cache slice.

Per-core kernel layout (one SPMD Bass program, data differs per core):
  S_T[tk, q] = (K_tile)^T-style scores with q streaming (N=512 supertiles)
  exp on ACT in batches of <=3 k-tiles (one PSUM-wide activation)
  staircase masking applied multiplicatively after exp (fp16)
  PV uses P_T chunks as stationary operand; V carries an extra ones
  column so the softmax denominator accumulates in PSUM alongside O.
"""

import os
import numpy as np

T = 4096
H = 16
HKV = 4
D = 128
BL = 32
NUM_SLOTS = 8192
SCALE = 0.08838834764831845
NCORES = 8
QH = 2                    # query heads per core
QSUP = 512                # queries per supertile (fp32 matmul N max)
NSUP = T // QSUP          # 8
KTILE = 128
NKT = T // KTILE          # 32
KBATCH = 3                # k-tiles per exp batch (PSUM: 2*3 + 2 banks)
DV = D + 1                # V width incl. ones column

_PROG_CACHE = {}
LAST_RESULT = None


def _plan_cache(slot_mapping):
    """Coalesce the cache scatter into contiguous row-range copies.

    Returns segments (dst_start, src_start, n, from_new): from_new rows come
    from the new k/v rows, others pass through the input cache.
    """
    sm = np.asarray(slot_mapping).astype(np.int64)
    src_of = np.full(NUM_SLOTS, -1, np.int64)
    src_of[sm] = np.arange(sm.shape[0])
    segs = []
    r = 0
    while r < NUM_SLOTS:
        if src_of[r] < 0:
            r2 = r
            while r2 < NUM_SLOTS and src_of[r2] < 0:
                r2 += 1
            segs.append((r, r, r2 - r, False))
            r = r2
        else:
            r2 = r
            while r2 + 1 < NUM_SLOTS and src_of[r2 + 1] == src_of[r2] + 1:
                r2 += 1
            segs.append((r, int(src_of[r]), r2 - r + 1, True))
            r = r2 + 1
    return tuple(segs)


def _maybe_patch_ldwopt():
    if not os.environ.get("KNL_LDWOPT"):
        return
    import concourse.bass_utils as bu

    if getattr(bu, "_knl_ldwopt_patched", False):
        return
    orig = bu.run_command

    def patched(cmd, *a, **kw):
        cmd = ["--enable-ldw-opt=true" if c == "--enable-ldw-opt=false" else c
               for c in cmd]
        return orig(cmd, *a, **kw)

    bu.run_command = patched
    bu._knl_ldwopt_patched = True


def _build_program(plan):
    import concourse.mybir as mybir
    from concourse import bacc
    from concourse.tile import TileContext

    _maybe_patch_ldwopt()
    f32 = mybir.dt.float32
    f32r = mybir.dt.float32r
    f16 = mybir.dt.float16
    qk = os.environ.get("KNL_QK", "f16")
    if qk == "bf16":
        fqk = mybir.dt.bfloat16
    elif qk == "f32r":
        fqk = f32r
    else:
        fqk = f16
    EXP = mybir.ActivationFunctionType.Exp

    nc = bacc.Bacc("TRN2", target_bir_lowering=False, debug=False,
                   num_devices=NCORES)

    qT = nc.declare_dram_parameter("qT", [QH, 128, T], fqk, isOutput=False)
    kT = nc.declare_dram_parameter("kT", [128, T], fqk, isOutput=False)
    vp = nc.declare_dram_parameter("vp", [128, NKT * DV], f16, isOutput=False)
    mk = nc.declare_dram_parameter("mk", [128, 128], f16, isOutput=False)
    cin = nc.declare_dram_parameter("cin", [NUM_SLOTS, D], f32, isOutput=False)
    src = nc.declare_dram_parameter("src", [T, D], f32, isOutput=False)
    o_part = nc.declare_dram_parameter("o_part", [QH, T, D], f32, isOutput=True)
    cout = nc.declare_dram_parameter("cout", [NUM_SLOTS, D], f32, isOutput=True)

    with TileContext(nc) as tc:
        with tc.tile_pool(name="const", bufs=1) as cpool, \
             tc.tile_pool(name="work", bufs=4) as wpool, \
             tc.tile_pool(name="stp", bufs=2, space="PSUM") as stpool, \
             tc.tile_pool(name="opsum", bufs=1, space="PSUM") as opool, \
             tc.tile_pool(name="outp", bufs=8) as outpool:

            qT_sb = cpool.tile([128, QH * T], fqk, tag="qT_sb", name="qT_sb")
            kT_sb = cpool.tile([128, T], fqk, tag="kT_sb", name="kT_sb")
            vp_sb = cpool.tile([128, NKT * DV], f16, tag="vp_sb", name="vp_sb")
            mk_sb = cpool.tile([128, 128], f16, tag="mk_sb", name="mk_sb")

            # Pull the ACT exp-table load (~2.7us) into the DMA wait
            # window via a tiny dummy activation on a fresh tile.
            warm = wpool.tile([1, 1], f32, tag="warm", name="warm", bufs=1)
            nc.vector.memset(warm, 0.0)
            nc.scalar.activation(out=warm, in_=warm, func=EXP)

            # Loads, most urgent first (h0/s0 needs kT[0:512], qT h0 s0,
            # vp tiles 0..3, masks).
            nc.sync.dma_start(out=kT_sb[:, 0:128], in_=kT[:, 0:128])
            nc.gpsimd.dma_start(out=qT_sb[:, 0:QSUP], in_=qT[0, :, 0:QSUP])
            nc.sync.dma_start(out=kT_sb[:, 128:512], in_=kT[:, 128:512])
            nc.gpsimd.dma_start(out=vp_sb[:, 0:8 * DV], in_=vp[:, 0:8 * DV])
            nc.gpsimd.dma_start(out=mk_sb[:, :], in_=mk[:, :])
            for j in range(1, 8):
                nc.sync.dma_start(out=kT_sb[:, j * 512:(j + 1) * 512],
                                  in_=kT[:, j * 512:(j + 1) * 512])
            for j in range(1, 4):
                nc.sync.dma_start(out=vp_sb[:, j * 8 * DV:(j + 1) * 8 * DV],
                                  in_=vp[:, j * 8 * DV:(j + 1) * 8 * DV])
            for h in range(QH):
                for s in range(NSUP):
                    if h == 0 and s == 0:
                        continue
                    off = h * T + s * QSUP
                    nc.sync.dma_start(out=qT_sb[:, off:off + QSUP],
                                      in_=qT[h, :, s * QSUP:(s + 1) * QSUP])

            pending = []

            # Diagonal k-tiles only need q >= o*128 (o = in-supertile
            # offset): pack the four restricted-width score tiles into one
            # contiguous PSUM span, ordered so no matmul output crosses a
            # 2KB bank boundary.
            DIAG_SEG = {0: 0, 1: 512, 3: 896, 2: 1024}
            DIAG_W = {0: 512, 1: 384, 2: 256, 3: 128}
            DIAG_TOT = 1280

            def emit_pv(kind, pt, o01, o23, s, extra):
                for ki, c, lcol in _pv_iter(kind, s, extra):
                    ot = o01 if c < 2 else o23
                    col = (c % 2) * DV
                    if kind == "nd":
                        start = (ki == 0 and c % 2 == 0)
                        stop = False
                    else:
                        o = ki - 4 * s
                        start = (s == 0 and o == 0 and c % 2 == 0)
                        stop = (o == 1 and c == 1) or (o == 3 and c == 3)
                    nc.tensor.matmul(
                        ot[:, col:col + DV],
                        lhsT=pt[:, lcol:lcol + 128],
                        rhs=vp_sb[:, ki * DV:(ki + 1) * DV],
                        start=start, stop=stop,
                    )

            def _pv_iter(kind, s, extra):
                if kind == "nd":
                    for j, ki in enumerate(extra):
                        for c in range(4):
                            yield ki, c, j * QSUP + c * 128
                else:
                    for o in range(4):
                        for c in range(o, 4):
                            yield 4 * s + o, c, DIAG_SEG[o] + (c - o) * 128

            def flush(depth=2):
                while len(pending) > depth:
                    kind, pt, o01, o23, s, h, extra, last = pending.pop(0)
                    _emit_one(kind, pt, o01, o23, s, h, extra, last)

            def _emit_one(kind, pt, o01, o23, s, h, extra, last):
                emit_pv(kind, pt, o01, o23, s, extra)
                if last:
                    for c in range(4):
                        ot = o01 if c < 2 else o23
                        col = (c % 2) * DV
                        rc = outpool.tile([128, 1], f32, tag="rc", name="rc")
                        nc.vector.reciprocal(out=rc, in_=ot[:, col + D:col + DV])
                        osb = outpool.tile([128, D], f32, tag="osb", name="osb")
                        nc.vector.tensor_scalar_mul(osb, ot[:, col:col + D], rc)
                        r0 = s * QSUP + c * 128
                        nc.gpsimd.dma_start(
                            out=o_part[h, r0:r0 + 128, :], in_=osb)

            for h in range(QH):
                for s in range(NSUP):
                    o01 = opool.tile([128, 2 * DV], f32, tag="o01", name="o01")
                    o23 = opool.tile([128, 2 * DV], f32, tag="o23", name="o23")
                    nnd = 4 * s           # non-diagonal k-tiles
                    qoff = h * T + s * QSUP
                    for b0 in range(0, nnd, KBATCH):
                        batch = list(range(b0, min(b0 + KBATCH, nnd)))
                        nb = len(batch)
                        st = stpool.tile([128, nb * QSUP], f32, tag="st", name="st")
                        for j, ki in enumerate(batch):
                            nc.tensor.matmul(
                                st[:, j * QSUP:(j + 1) * QSUP],
                                lhsT=kT_sb[:, ki * 128:(ki + 1) * 128],
                                rhs=qT_sb[:, qoff:qoff + QSUP],
                                start=True, stop=True,
                            )
                        pt = wpool.tile([128, nb * QSUP], f16, tag="pt", name="pt")
                        nc.scalar.activation(out=pt, in_=st, func=EXP)
                        pending.append(("nd", pt, o01, o23, s, h, batch, False))
                        flush()
                    # diagonal batch: restricted q ranges, one exp
                    st = stpool.tile([128, DIAG_TOT], f32, tag="st", name="st")
                    for o in range(4):
                        ki = 4 * s + o
                        seg, w = DIAG_SEG[o], DIAG_W[o]
                        nc.tensor.matmul(
                            st[:, seg:seg + w],
                            lhsT=kT_sb[:, ki * 128:(ki + 1) * 128],
                            rhs=qT_sb[:, qoff + o * 128:qoff + QSUP],
                            start=True, stop=True,
                        )
                    pt = wpool.tile([128, DIAG_TOT], f16, tag="ptd",
                                    name="ptd", bufs=2)
                    nc.scalar.activation(out=pt, in_=st, func=EXP)
                    for o in range(4):
                        sl = pt[:, DIAG_SEG[o]:DIAG_SEG[o] + 128]
                        nc.vector.tensor_mul(sl, sl, mk_sb[:, 0:128])
                    pending.append(("dg", pt, o01, o23, s, h, None, True))
                    flush()
            flush(depth=0)

            # Cache update: pure DRAM->DRAM copies, chunked <=1024 rows.
            for (dst0, src0, n, from_new) in plan:
                s_t = src if from_new else cin
                for off in range(0, n, 1024):
                    m = min(1024, n - off)
                    nc.sync.dma_start(
                        out=cout[dst0 + off:dst0 + off + m, :],
                        in_=s_t[src0 + off:src0 + off + m, :])

    nc.compile()
    return nc


def _get_program(plan):
    if plan not in _PROG_CACHE:
        _PROG_CACHE[plan] = _build_program(plan)
    return _PROG_CACHE[plan]


def _make_masks():
    tk = np.arange(128)[:, None] // BL          # [128,1] 0..3
    ql = np.arange(128)[None, :] // BL          # [1,128] 0..3
    return (tk <= ql).astype(np.float16)        # [128, 128] local staircase


def kernel(q, k, v, k_cache, v_cache, slot_mapping, block_length):
    global LAST_RESULT
    from concourse.bass_utils import run_bass_kernel_spmd

    q = np.ascontiguousarray(np.asarray(q, dtype=np.float32))
    k = np.ascontiguousarray(np.asarray(k, dtype=np.float32))
    v = np.ascontiguousarray(np.asarray(v, dtype=np.float32))
    k_cache = np.ascontiguousarray(np.asarray(k_cache, dtype=np.float32))
    v_cache = np.ascontiguousarray(np.asarray(v_cache, dtype=np.float32))
    sm = np.asarray(slot_mapping).astype(np.int64)
    assert int(block_length) == BL
    assert q.shape == (T, H * D) and k.shape == (T, HKV * D)

    plan = _plan_cache(sm)
    nc = _get_program(plan)
    qk = os.environ.get("KNL_QK", "f16")
    if qk == "bf16":
        import ml_dtypes
        qk_np = ml_dtypes.bfloat16
    elif qk == "f32r":
        qk_np = np.float32
    else:
        qk_np = np.float16

    qh = q.reshape(T, H, D)
    kh = k.reshape(T, HKV, D)
    vh = v.reshape(T, HKV, D)
    kch = k_cache.reshape(NUM_SLOTS, HKV, D)
    vch = v_cache.reshape(NUM_SLOTS, HKV, D)
    mk = _make_masks()

    in_maps = []
    for c in range(NCORES):
        g = c // 2
        qTc = np.ascontiguousarray(
            (qh[:, 2 * c:2 * c + 2, :] * SCALE).transpose(1, 2, 0)).astype(qk_np)
        kTc = np.ascontiguousarray(kh[:, g, :].T).astype(qk_np)
        vpc = np.ones((T, DV), np.float16)
        vpc[:, :D] = vh[:, g, :].astype(np.float16)
        vpc = np.ascontiguousarray(
            vpc.reshape(NKT, 128, DV).transpose(1, 0, 2).reshape(128, NKT * DV))
        if c % 2 == 0:
            cin = np.ascontiguousarray(kch[:, g, :])
            srcr = np.ascontiguousarray(kh[:, g, :])
        else:
            cin = np.ascontiguousarray(vch[:, g, :])
            srcr = np.ascontiguousarray(vh[:, g, :])
        in_maps.append({"qT": qTc, "kT": kTc, "vp": vpc, "mk": mk,
                        "cin": cin, "src": srcr})

    res = run_bass_kernel_spmd(nc, in_maps, list(range(NCORES)),
                               trace=bool(os.environ.get("KNL_TRACE")))
    LAST_RESULT = res

    o = np.empty((T, H, D), np.float32)
    for c in range(NCORES):
        op = res.results[c]["o_part"]          # [QH, T, D]
        o[:, 2 * c, :] = op[0]
        o[:, 2 * c + 1, :] = op[1]
    o = o.reshape(T, H * D)
    kc = np.empty((NUM_SLOTS, HKV * D), np.float32)
    vc = np.empty((NUM_SLOTS, HKV * D), np.float32)
    for c in range(NCORES):
        g = c // 2
        dst = kc if c % 2 == 0 else vc
        dst[:, g * D:(g + 1) * D] = res.results[c]["cout"]
    return o, kc, vc


# revision 12
# speedup vs baseline: 1.0173x; 1.0173x over previous
"""BlockAttention prefill kernel for Trainium2, 8-core tensor-parallel.

Reference op (see problem): scatter K/V rows into paged caches, then
block-causal (staircase, block_length=32) attention over T=4096 tokens,
16 query heads / 4 KV heads (GQA), head_dim=128, fp32.

Sharding: pure tensor parallelism over heads. Core c computes query heads
{2c, 2c+1}, which share KV head c//2. Cache update is split so core 2j
produces the K-cache slice of KV head j and core 2j+1 the V-cache slice.

Per-core kernel layout (one SPMD Bass program, data differs per core):
  S_T[tk, q] = (K_tile)^T-style scores with q streaming (N=512 supertiles)
  exp on ACT in batches of <=3 k-tiles (one PSUM-wide activation)
  staircase masking applied multiplicatively after exp (fp16)
  PV uses P_T chunks as stationary operand; V carries an extra ones
  column so the softmax denominator accumulates in PSUM alongside O.
"""

import os
import numpy as np

T = 4096
H = 16
HKV = 4
D = 128
BL = 32
NUM_SLOTS = 8192
SCALE = 0.08838834764831845
NCORES = 8
QH = 2                    # query heads per core
QSUP = 512                # queries per supertile (fp32 matmul N max)
NSUP = T // QSUP          # 8
KTILE = 128
NKT = T // KTILE          # 32
KBATCH = 3                # k-tiles per exp batch (PSUM: 2*3 + 2 banks)
DV = D + 1                # V width incl. ones column

_PROG_CACHE = {}
LAST_RESULT = None


def _plan_cache(slot_mapping):
    """Coalesce the cache scatter into contiguous row-range copies.

    Returns segments (dst_start, src_start, n, from_new): from_new rows come
    from the new k/v rows, others pass through the input cache.
    """
    sm = np.asarray(slot_mapping).astype(np.int64)
    src_of = np.full(NUM_SLOTS, -1, np.int64)
    src_of[sm] = np.arange(sm.shape[0])
    segs = []
    r = 0
    while r < NUM_SLOTS:
        if src_of[r] < 0:
            r2 = r
            while r2 < NUM_SLOTS and src_of[r2] < 0:
                r2 += 1
            segs.append((r, r, r2 - r, False))
            r = r2
        else:
            r2 = r
            while r2 + 1 < NUM_SLOTS and src_of[r2 + 1] == src_of[r2] + 1:
                r2 += 1
            segs.append((r, int(src_of[r]), r2 - r + 1, True))
            r = r2 + 1
    return tuple(segs)


def _maybe_patch_ldwopt():
    if not os.environ.get("KNL_LDWOPT"):
        return
    import concourse.bass_utils as bu

    if getattr(bu, "_knl_ldwopt_patched", False):
        return
    orig = bu.run_command

    def patched(cmd, *a, **kw):
        cmd = ["--enable-ldw-opt=true" if c == "--enable-ldw-opt=false" else c
               for c in cmd]
        return orig(cmd, *a, **kw)

    bu.run_command = patched
    bu._knl_ldwopt_patched = True


def _build_program(plan):
    import concourse.mybir as mybir
    from concourse import bacc
    from concourse.tile import TileContext

    _maybe_patch_ldwopt()
    f32 = mybir.dt.float32
    f32r = mybir.dt.float32r
    f16 = mybir.dt.float16
    qk = os.environ.get("KNL_QK", "f16")
    if qk == "bf16":
        fqk = mybir.dt.bfloat16
    elif qk == "f32r":
        fqk = f32r
    else:
        fqk = f16
    EXP = mybir.ActivationFunctionType.Exp

    nc = bacc.Bacc("TRN2", target_bir_lowering=False, debug=False,
                   num_devices=NCORES)

    qT = nc.declare_dram_parameter("qT", [QH, 128, T], fqk, isOutput=False)
    kT = nc.declare_dram_parameter("kT", [128, T], fqk, isOutput=False)
    vp = nc.declare_dram_parameter("vp", [128, NKT * DV], f16, isOutput=False)
    mk = nc.declare_dram_parameter("mk", [128, 128], f16, isOutput=False)
    cin = nc.declare_dram_parameter("cin", [NUM_SLOTS, D], f32, isOutput=False)
    src = nc.declare_dram_parameter("src", [T, D], f32, isOutput=False)
    o_part = nc.declare_dram_parameter("o_part", [QH, T, D], f32, isOutput=True)
    cout = nc.declare_dram_parameter("cout", [NUM_SLOTS, D], f32, isOutput=True)

    with TileContext(nc) as tc:
        with tc.tile_pool(name="const", bufs=1) as cpool, \
             tc.tile_pool(name="work", bufs=4) as wpool, \
             tc.tile_pool(name="stp", bufs=2, space="PSUM") as stpool, \
             tc.tile_pool(name="opsum", bufs=1, space="PSUM") as opool, \
             tc.tile_pool(name="outp", bufs=8) as outpool:

            qT_sb = cpool.tile([128, QH * T], fqk, tag="qT_sb", name="qT_sb")
            kT_sb = cpool.tile([128, T], fqk, tag="kT_sb", name="kT_sb")
            vp_sb = cpool.tile([128, NKT * DV], f16, tag="vp_sb", name="vp_sb")
            mk_sb = cpool.tile([128, 128], f16, tag="mk_sb", name="mk_sb")

            # Pull the ACT exp-table load (~2.7us) into the DMA wait
            # window via a tiny dummy activation on a fresh tile.
            warm = wpool.tile([1, 1], f32, tag="warm", name="warm", bufs=1)
            nc.vector.memset(warm, 0.0)
            nc.scalar.activation(out=warm, in_=warm, func=EXP)

            # Loads, most urgent first (h0/s0 needs kT[0:512], qT h0 s0,
            # vp tiles 0..3, masks).
            nc.sync.dma_start(out=kT_sb[:, 0:128], in_=kT[:, 0:128])
            nc.gpsimd.dma_start(out=qT_sb[:, 0:QSUP], in_=qT[0, :, 0:QSUP])
            nc.sync.dma_start(out=kT_sb[:, 128:512], in_=kT[:, 128:512])
            nc.gpsimd.dma_start(out=vp_sb[:, 0:8 * DV], in_=vp[:, 0:8 * DV])
            nc.gpsimd.dma_start(out=mk_sb[:, :], in_=mk[:, :])
            for j in range(1, 8):
                nc.sync.dma_start(out=kT_sb[:, j * 512:(j + 1) * 512],
                                  in_=kT[:, j * 512:(j + 1) * 512])
            for j in range(1, 4):
                nc.sync.dma_start(out=vp_sb[:, j * 8 * DV:(j + 1) * 8 * DV],
                                  in_=vp[:, j * 8 * DV:(j + 1) * 8 * DV])
            for h in range(QH):
                for s in range(NSUP):
                    if h == 0 and s == 0:
                        continue
                    off = h * T + s * QSUP
                    nc.sync.dma_start(out=qT_sb[:, off:off + QSUP],
                                      in_=qT[h, :, s * QSUP:(s + 1) * QSUP])

            pending = []

            # Diagonal k-tiles only need q >= o*128 (o = in-supertile
            # offset): pack the four restricted-width score tiles into one
            # contiguous PSUM span, ordered so no matmul output crosses a
            # 2KB bank boundary.
            DIAG_SEG = {0: 0, 1: 512, 3: 896, 2: 1024}
            DIAG_W = {0: 512, 1: 384, 2: 256, 3: 128}
            DIAG_TOT = 1280

            def emit_pv(kind, pt, o01, o23, s, extra):
                for ki, c, lcol in _pv_iter(kind, s, extra):
                    ot = o01 if c < 2 else o23
                    col = (c % 2) * DV
                    if kind == "nd":
                        start = (ki == 0 and c % 2 == 0)
                        stop = False
                    else:
                        o = ki - 4 * s
                        start = (s == 0 and o == 0 and c % 2 == 0)
                        stop = (o == 1 and c == 1) or (o == 3 and c == 3)
                    nc.tensor.matmul(
                        ot[:, col:col + DV],
                        lhsT=pt[:, lcol:lcol + 128],
                        rhs=vp_sb[:, ki * DV:(ki + 1) * DV],
                        start=start, stop=stop,
                    )

            def _pv_iter(kind, s, extra):
                if kind == "nd":
                    for j, ki in enumerate(extra):
                        for c in range(4):
                            yield ki, c, j * QSUP + c * 128
                else:
                    for o in range(4):
                        for c in range(o, 4):
                            yield 4 * s + o, c, DIAG_SEG[o] + (c - o) * 128

            def flush(depth=2):
                while len(pending) > depth:
                    kind, pt, o01, o23, s, h, extra, last = pending.pop(0)
                    _emit_one(kind, pt, o01, o23, s, h, extra, last)

            def _emit_one(kind, pt, o01, o23, s, h, extra, last):
                emit_pv(kind, pt, o01, o23, s, extra)
                if last:
                    for c in range(4):
                        ot = o01 if c < 2 else o23
                        col = (c % 2) * DV
                        rc = outpool.tile([128, 1], f32, tag="rc", name="rc")
                        nc.vector.reciprocal(out=rc, in_=ot[:, col + D:col + DV])
                        osb = outpool.tile([128, D], f32, tag="osb", name="osb")
                        nc.vector.tensor_scalar_mul(osb, ot[:, col:col + D], rc)
                        r0 = s * QSUP + c * 128
                        nc.sync.dma_start(
                            out=o_part[h, r0:r0 + 128, :], in_=osb)

            for h in range(QH):
                for s in range(NSUP):
                    o01 = opool.tile([128, 2 * DV], f32, tag="o01", name="o01")
                    o23 = opool.tile([128, 2 * DV], f32, tag="o23", name="o23")
                    nnd = 4 * s           # non-diagonal k-tiles
                    qoff = h * T + s * QSUP
                    for b0 in range(0, nnd, KBATCH):
                        batch = list(range(b0, min(b0 + KBATCH, nnd)))
                        nb = len(batch)
                        st = stpool.tile([128, nb * QSUP], f32, tag="st", name="st")
                        for j, ki in enumerate(batch):
                            nc.tensor.matmul(
                                st[:, j * QSUP:(j + 1) * QSUP],
                                lhsT=kT_sb[:, ki * 128:(ki + 1) * 128],
                                rhs=qT_sb[:, qoff:qoff + QSUP],
                                start=True, stop=True,
                            )
                        pt = wpool.tile([128, nb * QSUP], f16, tag="pt", name="pt")
                        nc.scalar.activation(out=pt, in_=st, func=EXP)
                        pending.append(("nd", pt, o01, o23, s, h, batch, False))
                        flush()
                    # diagonal batch: restricted q ranges, one exp
                    st = stpool.tile([128, DIAG_TOT], f32, tag="st", name="st")
                    for o in range(4):
                        ki = 4 * s + o
                        seg, w = DIAG_SEG[o], DIAG_W[o]
                        nc.tensor.matmul(
                            st[:, seg:seg + w],
                            lhsT=kT_sb[:, ki * 128:(ki + 1) * 128],
                            rhs=qT_sb[:, qoff + o * 128:qoff + QSUP],
                            start=True, stop=True,
                        )
                    pt = wpool.tile([128, DIAG_TOT], f16, tag="ptd",
                                    name="ptd", bufs=2)
                    nc.scalar.activation(out=pt, in_=st, func=EXP)
                    for o in range(4):
                        sl = pt[:, DIAG_SEG[o]:DIAG_SEG[o] + 128]
                        nc.vector.tensor_mul(sl, sl, mk_sb[:, 0:128])
                    pending.append(("dg", pt, o01, o23, s, h, None, True))
                    flush()
            flush(depth=0)

            # Cache update: pure DRAM->DRAM copies, chunked <=1024 rows.
            for (dst0, src0, n, from_new) in plan:
                s_t = src if from_new else cin
                for off in range(0, n, 1024):
                    m = min(1024, n - off)
                    nc.sync.dma_start(
                        out=cout[dst0 + off:dst0 + off + m, :],
                        in_=s_t[src0 + off:src0 + off + m, :])

    nc.compile()
    return nc


def _get_program(plan):
    if plan not in _PROG_CACHE:
        _PROG_CACHE[plan] = _build_program(plan)
    return _PROG_CACHE[plan]


def _make_masks():
    tk = np.arange(128)[:, None] // BL          # [128,1] 0..3
    ql = np.arange(128)[None, :] // BL          # [1,128] 0..3
    return (tk <= ql).astype(np.float16)        # [128, 128] local staircase


def kernel(q, k, v, k_cache, v_cache, slot_mapping, block_length):
    global LAST_RESULT
    from concourse.bass_utils import run_bass_kernel_spmd

    q = np.ascontiguousarray(np.asarray(q, dtype=np.float32))
    k = np.ascontiguousarray(np.asarray(k, dtype=np.float32))
    v = np.ascontiguousarray(np.asarray(v, dtype=np.float32))
    k_cache = np.ascontiguousarray(np.asarray(k_cache, dtype=np.float32))
    v_cache = np.ascontiguousarray(np.asarray(v_cache, dtype=np.float32))
    sm = np.asarray(slot_mapping).astype(np.int64)
    assert int(block_length) == BL
    assert q.shape == (T, H * D) and k.shape == (T, HKV * D)

    plan = _plan_cache(sm)
    nc = _get_program(plan)
    qk = os.environ.get("KNL_QK", "f16")
    if qk == "bf16":
        import ml_dtypes
        qk_np = ml_dtypes.bfloat16
    elif qk == "f32r":
        qk_np = np.float32
    else:
        qk_np = np.float16

    qh = q.reshape(T, H, D)
    kh = k.reshape(T, HKV, D)
    vh = v.reshape(T, HKV, D)
    kch = k_cache.reshape(NUM_SLOTS, HKV, D)
    vch = v_cache.reshape(NUM_SLOTS, HKV, D)
    mk = _make_masks()

    in_maps = []
    for c in range(NCORES):
        g = c // 2
        qTc = np.ascontiguousarray(
            (qh[:, 2 * c:2 * c + 2, :] * SCALE).transpose(1, 2, 0)).astype(qk_np)
        kTc = np.ascontiguousarray(kh[:, g, :].T).astype(qk_np)
        vpc = np.ones((T, DV), np.float16)
        vpc[:, :D] = vh[:, g, :].astype(np.float16)
        vpc = np.ascontiguousarray(
            vpc.reshape(NKT, 128, DV).transpose(1, 0, 2).reshape(128, NKT * DV))
        if c % 2 == 0:
            cin = np.ascontiguousarray(kch[:, g, :])
            srcr = np.ascontiguousarray(kh[:, g, :])
        else:
            cin = np.ascontiguousarray(vch[:, g, :])
            srcr = np.ascontiguousarray(vh[:, g, :])
        in_maps.append({"qT": qTc, "kT": kTc, "vp": vpc, "mk": mk,
                        "cin": cin, "src": srcr})

    res = run_bass_kernel_spmd(nc, in_maps, list(range(NCORES)),
                               trace=bool(os.environ.get("KNL_TRACE")))
    LAST_RESULT = res

    o = np.empty((T, H, D), np.float32)
    for c in range(NCORES):
        op = res.results[c]["o_part"]          # [QH, T, D]
        o[:, 2 * c, :] = op[0]
        o[:, 2 * c + 1, :] = op[1]
    o = o.reshape(T, H * D)
    kc = np.empty((NUM_SLOTS, HKV * D), np.float32)
    vc = np.empty((NUM_SLOTS, HKV * D), np.float32)
    for c in range(NCORES):
        g = c // 2
        dst = kc if c % 2 == 0 else vc
        dst[:, g * D:(g + 1) * D] = res.results[c]["cout"]
    return o, kc, vc


# revision 13
# speedup vs baseline: 1.0182x; 1.0009x over previous
"""BlockAttention prefill kernel for Trainium2, 8-core tensor-parallel.

Reference op (see problem): scatter K/V rows into paged caches, then
block-causal (staircase, block_length=32) attention over T=4096 tokens,
16 query heads / 4 KV heads (GQA), head_dim=128, fp32.

Sharding: pure tensor parallelism over heads. Core c computes query heads
{2c, 2c+1}, which share KV head c//2. Cache update is split so core 2j
produces the K-cache slice of KV head j and core 2j+1 the V-cache slice.

Per-core kernel layout (one SPMD Bass program, data differs per core):
  S_T[tk, q] = (K_tile)^T-style scores with q streaming (N=512 supertiles)
  exp on ACT in batches of <=3 k-tiles (one PSUM-wide activation)
  staircase masking applied multiplicatively after exp (fp16)
  PV uses P_T chunks as stationary operand; V carries an extra ones
  column so the softmax denominator accumulates in PSUM alongside O.
"""

import os
import numpy as np

T = 4096
H = 16
HKV = 4
D = 128
BL = 32
NUM_SLOTS = 8192
SCALE = 0.08838834764831845
NCORES = 8
QH = 2                    # query heads per core
QSUP = 512                # queries per supertile (fp32 matmul N max)
NSUP = T // QSUP          # 8
KTILE = 128
NKT = T // KTILE          # 32
KBATCH = 3                # k-tiles per exp batch (PSUM: 2*3 + 2 banks)
DV = D + 1                # V width incl. ones column

_PROG_CACHE = {}
LAST_RESULT = None


def _plan_cache(slot_mapping):
    """Coalesce the cache scatter into contiguous row-range copies.

    Returns segments (dst_start, src_start, n, from_new): from_new rows come
    from the new k/v rows, others pass through the input cache.
    """
    sm = np.asarray(slot_mapping).astype(np.int64)
    src_of = np.full(NUM_SLOTS, -1, np.int64)
    src_of[sm] = np.arange(sm.shape[0])
    segs = []
    r = 0
    while r < NUM_SLOTS:
        if src_of[r] < 0:
            r2 = r
            while r2 < NUM_SLOTS and src_of[r2] < 0:
                r2 += 1
            segs.append((r, r, r2 - r, False))
            r = r2
        else:
            r2 = r
            while r2 + 1 < NUM_SLOTS and src_of[r2 + 1] == src_of[r2] + 1:
                r2 += 1
            segs.append((r, int(src_of[r]), r2 - r + 1, True))
            r = r2 + 1
    return tuple(segs)


def _maybe_patch_ldwopt():
    if not os.environ.get("KNL_LDWOPT"):
        return
    import concourse.bass_utils as bu

    if getattr(bu, "_knl_ldwopt_patched", False):
        return
    orig = bu.run_command

    def patched(cmd, *a, **kw):
        cmd = ["--enable-ldw-opt=true" if c == "--enable-ldw-opt=false" else c
               for c in cmd]
        return orig(cmd, *a, **kw)

    bu.run_command = patched
    bu._knl_ldwopt_patched = True


def _build_program(plan):
    import concourse.mybir as mybir
    from concourse import bacc
    from concourse.tile import TileContext

    _maybe_patch_ldwopt()
    f32 = mybir.dt.float32
    f32r = mybir.dt.float32r
    f16 = mybir.dt.float16
    qk = os.environ.get("KNL_QK", "f16")
    if qk == "bf16":
        fqk = mybir.dt.bfloat16
    elif qk == "f32r":
        fqk = f32r
    else:
        fqk = f16
    EXP = mybir.ActivationFunctionType.Exp

    nc = bacc.Bacc("TRN2", target_bir_lowering=False, debug=False,
                   num_devices=NCORES)

    qT = nc.declare_dram_parameter("qT", [QH, 128, T], fqk, isOutput=False)
    kT = nc.declare_dram_parameter("kT", [128, T], fqk, isOutput=False)
    vp = nc.declare_dram_parameter("vp", [128, NKT * DV], f16, isOutput=False)
    mk = nc.declare_dram_parameter("mk", [128, 128], f16, isOutput=False)
    cin = nc.declare_dram_parameter("cin", [NUM_SLOTS, D], f32, isOutput=False)
    src = nc.declare_dram_parameter("src", [T, D], f32, isOutput=False)
    o_part = nc.declare_dram_parameter("o_part", [QH, T, D], f32, isOutput=True)
    cout = nc.declare_dram_parameter("cout", [NUM_SLOTS, D], f32, isOutput=True)

    with TileContext(nc) as tc:
        with tc.tile_pool(name="const", bufs=1) as cpool, \
             tc.tile_pool(name="work", bufs=4) as wpool, \
             tc.tile_pool(name="stp", bufs=2, space="PSUM") as stpool, \
             tc.tile_pool(name="opsum", bufs=1, space="PSUM") as opool, \
             tc.tile_pool(name="outp", bufs=8) as outpool:

            qT_sb = cpool.tile([128, QH * T], fqk, tag="qT_sb", name="qT_sb")
            kT_sb = cpool.tile([128, T], fqk, tag="kT_sb", name="kT_sb")
            vp_sb = cpool.tile([128, NKT * DV], f16, tag="vp_sb", name="vp_sb")
            mk_sb = cpool.tile([128, 128], f16, tag="mk_sb", name="mk_sb")

            # Pull the ACT exp-table load (~2.7us) into the DMA wait
            # window via a tiny dummy activation on a fresh tile.
            warm = wpool.tile([1, 1], f32, tag="warm", name="warm", bufs=1)
            nc.vector.memset(warm, 0.0)
            nc.scalar.activation(out=warm, in_=warm, func=EXP)

            # Loads, most urgent first (h0/s0 needs kT[0:512], qT h0 s0,
            # vp tiles 0..3, masks).
            nc.sync.dma_start(out=kT_sb[:, 0:128], in_=kT[:, 0:128])
            nc.gpsimd.dma_start(out=qT_sb[:, 0:QSUP], in_=qT[0, :, 0:QSUP])
            nc.sync.dma_start(out=kT_sb[:, 128:512], in_=kT[:, 128:512])
            nc.gpsimd.dma_start(out=vp_sb[:, 0:8 * DV], in_=vp[:, 0:8 * DV])
            nc.gpsimd.dma_start(out=mk_sb[:, :], in_=mk[:, :])
            for j in range(1, 8):
                nc.sync.dma_start(out=kT_sb[:, j * 512:(j + 1) * 512],
                                  in_=kT[:, j * 512:(j + 1) * 512])
            for j in range(1, 4):
                nc.sync.dma_start(out=vp_sb[:, j * 8 * DV:(j + 1) * 8 * DV],
                                  in_=vp[:, j * 8 * DV:(j + 1) * 8 * DV])
            for h in range(QH):
                for s in range(NSUP):
                    if h == 0 and s == 0:
                        continue
                    off = h * T + s * QSUP
                    nc.sync.dma_start(out=qT_sb[:, off:off + QSUP],
                                      in_=qT[h, :, s * QSUP:(s + 1) * QSUP])

            pending = []

            # Diagonal k-tiles only need q >= o*128 (o = in-supertile
            # offset): pack the four restricted-width score tiles into one
            # contiguous PSUM span, ordered so no matmul output crosses a
            # 2KB bank boundary.
            DIAG_SEG = {0: 0, 1: 512, 3: 896, 2: 1024}
            DIAG_W = {0: 512, 1: 384, 2: 256, 3: 128}
            DIAG_TOT = 1280

            def emit_pv(kind, pt, o01, o23, s, extra):
                for ki, c, lcol in _pv_iter(kind, s, extra):
                    ot = o01 if c < 2 else o23
                    col = (c % 2) * DV
                    if kind == "nd":
                        start = (ki == 0 and c % 2 == 0)
                        stop = False
                    else:
                        o = ki - 4 * s
                        start = (s == 0 and o == 0 and c % 2 == 0)
                        stop = (o == 1 and c == 1) or (o == 3 and c == 3)
                    nc.tensor.matmul(
                        ot[:, col:col + DV],
                        lhsT=pt[:, lcol:lcol + 128],
                        rhs=vp_sb[:, ki * DV:(ki + 1) * DV],
                        start=start, stop=stop,
                    )

            def _pv_iter(kind, s, extra):
                if kind == "nd":
                    for j, ki in enumerate(extra):
                        for c in range(4):
                            yield ki, c, j * QSUP + c * 128
                else:
                    for o in range(4):
                        for c in range(o, 4):
                            yield 4 * s + o, c, DIAG_SEG[o] + (c - o) * 128

            def flush(depth=2):
                while len(pending) > depth:
                    kind, pt, o01, o23, s, h, extra, last = pending.pop(0)
                    _emit_one(kind, pt, o01, o23, s, h, extra, last)

            def _emit_one(kind, pt, o01, o23, s, h, extra, last):
                emit_pv(kind, pt, o01, o23, s, extra)
                if last:
                    for c in range(4):
                        ot = o01 if c < 2 else o23
                        col = (c % 2) * DV
                        rc = outpool.tile([128, 1], f32, tag="rc", name="rc")
                        nc.vector.reciprocal(out=rc, in_=ot[:, col + D:col + DV])
                        osb = outpool.tile([128, D], f32, tag="osb", name="osb")
                        nc.vector.tensor_scalar_mul(osb, ot[:, col:col + D], rc)
                        r0 = s * QSUP + c * 128
                        nc.sync.dma_start(
                            out=o_part[h, r0:r0 + 128, :], in_=osb)

            for h in range(QH):
                for s in range(NSUP):
                    o01 = opool.tile([128, 2 * DV], f32, tag="o01", name="o01")
                    o23 = opool.tile([128, 2 * DV], f32, tag="o23", name="o23")
                    nnd = 4 * s           # non-diagonal k-tiles
                    qoff = h * T + s * QSUP
                    for b0 in range(0, nnd, KBATCH):
                        batch = list(range(b0, min(b0 + KBATCH, nnd)))
                        nb = len(batch)
                        st = stpool.tile([128, nb * QSUP], f32, tag="st", name="st")
                        for j, ki in enumerate(batch):
                            nc.tensor.matmul(
                                st[:, j * QSUP:(j + 1) * QSUP],
                                lhsT=kT_sb[:, ki * 128:(ki + 1) * 128],
                                rhs=qT_sb[:, qoff:qoff + QSUP],
                                start=True, stop=True,
                            )
                        pt = wpool.tile([128, nb * QSUP], f16, tag="pt", name="pt")
                        nc.scalar.activation(out=pt, in_=st, func=EXP)
                        pending.append(("nd", pt, o01, o23, s, h, batch, False))
                        flush()
                    # diagonal batch: restricted q ranges, one exp
                    st = stpool.tile([128, DIAG_TOT], f32, tag="st", name="st")
                    for o in range(4):
                        ki = 4 * s + o
                        seg, w = DIAG_SEG[o], DIAG_W[o]
                        nc.tensor.matmul(
                            st[:, seg:seg + w],
                            lhsT=kT_sb[:, ki * 128:(ki + 1) * 128],
                            rhs=qT_sb[:, qoff + o * 128:qoff + QSUP],
                            start=True, stop=True,
                        )
                    pt = wpool.tile([128, DIAG_TOT], f16, tag="ptd",
                                    name="ptd", bufs=2)
                    nc.scalar.activation(out=pt, in_=st, func=EXP)
                    for o in range(4):
                        sl = pt[:, DIAG_SEG[o]:DIAG_SEG[o] + 128]
                        nc.vector.tensor_mul(sl, sl, mk_sb[:, 0:128])
                    pending.append(("dg", pt, o01, o23, s, h, None, True))
                    flush()
            flush(depth=0)

            # Cache update: pure DRAM->DRAM copies, chunked <=1024 rows.
            for (dst0, src0, n, from_new) in plan:
                s_t = src if from_new else cin
                for off in range(0, n, 1024):
                    m = min(1024, n - off)
                    nc.sync.dma_start(
                        out=cout[dst0 + off:dst0 + off + m, :],
                        in_=s_t[src0 + off:src0 + off + m, :])

    nc.compile()
    return nc


def _get_program(plan):
    if plan not in _PROG_CACHE:
        _PROG_CACHE[plan] = _build_program(plan)
    return _PROG_CACHE[plan]


def _make_masks():
    tk = np.arange(128)[:, None] // BL          # [128,1] 0..3
    ql = np.arange(128)[None, :] // BL          # [1,128] 0..3
    return (tk <= ql).astype(np.float16)        # [128, 128] local staircase


def kernel(q, k, v, k_cache, v_cache, slot_mapping, block_length):
    global LAST_RESULT
    from concourse.bass_utils import run_bass_kernel_spmd

    q = np.ascontiguousarray(np.asarray(q, dtype=np.float32))
    k = np.ascontiguousarray(np.asarray(k, dtype=np.float32))
    v = np.ascontiguousarray(np.asarray(v, dtype=np.float32))
    k_cache = np.ascontiguousarray(np.asarray(k_cache, dtype=np.float32))
    v_cache = np.ascontiguousarray(np.asarray(v_cache, dtype=np.float32))
    sm = np.asarray(slot_mapping).astype(np.int64)
    assert int(block_length) == BL
    assert q.shape == (T, H * D) and k.shape == (T, HKV * D)

    plan = _plan_cache(sm)
    nc = _get_program(plan)
    qk = os.environ.get("KNL_QK", "f16")
    if qk == "bf16":
        import ml_dtypes
        qk_np = ml_dtypes.bfloat16
    elif qk == "f32r":
        qk_np = np.float32
    else:
        qk_np = np.float16

    qh = q.reshape(T, H, D)
    kh = k.reshape(T, HKV, D)
    vh = v.reshape(T, HKV, D)
    kch = k_cache.reshape(NUM_SLOTS, HKV, D)
    vch = v_cache.reshape(NUM_SLOTS, HKV, D)
    mk = _make_masks()

    in_maps = []
    for c in range(NCORES):
        g = c // 2
        qTc = np.ascontiguousarray(
            (qh[:, 2 * c:2 * c + 2, :] * SCALE).transpose(1, 2, 0)).astype(qk_np)
        kTc = np.ascontiguousarray(kh[:, g, :].T).astype(qk_np)
        vpc = np.ones((T, DV), np.float16)
        vpc[:, :D] = vh[:, g, :].astype(np.float16)
        vpc = np.ascontiguousarray(
            vpc.reshape(NKT, 128, DV).transpose(1, 0, 2).reshape(128, NKT * DV))
        if c % 2 == 0:
            cin = np.ascontiguousarray(kch[:, g, :])
            srcr = np.ascontiguousarray(kh[:, g, :])
        else:
            cin = np.ascontiguousarray(vch[:, g, :])
            srcr = np.ascontiguousarray(vh[:, g, :])
        in_maps.append({"qT": qTc, "kT": kTc, "vp": vpc, "mk": mk,
                        "cin": cin, "src": srcr})

    trace = bool(os.environ.get("KNL_TRACE"))
    if trace:
        try:
            import antenv.axon_hooks  # noqa: F401
        except ImportError:
            trace = False
    res = run_bass_kernel_spmd(nc, in_maps, list(range(NCORES)), trace=trace)
    LAST_RESULT = res

    o = np.empty((T, H, D), np.float32)
    for c in range(NCORES):
        op = res.results[c]["o_part"]          # [QH, T, D]
        o[:, 2 * c, :] = op[0]
        o[:, 2 * c + 1, :] = op[1]
    o = o.reshape(T, H * D)
    kc = np.empty((NUM_SLOTS, HKV * D), np.float32)
    vc = np.empty((NUM_SLOTS, HKV * D), np.float32)
    for c in range(NCORES):
        g = c // 2
        dst = kc if c % 2 == 0 else vc
        dst[:, g * D:(g + 1) * D] = res.results[c]["cout"]
    return o, kc, vc


# revision 14
# speedup vs baseline: 1.0316x; 1.0132x over previous
"""BlockAttention prefill kernel for Trainium2, 8-core tensor-parallel.

Reference op (see problem): scatter K/V rows into paged caches, then
block-causal (staircase, block_length=32) attention over T=4096 tokens,
16 query heads / 4 KV heads (GQA), head_dim=128, fp32.

Sharding: pure tensor parallelism over heads. Core c computes query heads
{2c, 2c+1}, which share KV head c//2. Cache update is split so core 2j
produces the K-cache slice of KV head j and core 2j+1 the V-cache slice.

Per-core kernel layout (one SPMD Bass program, data differs per core):
  S_T[tk, q] = (K_tile)^T-style scores with q streaming (N=512 supertiles)
  exp on ACT in batches of <=3 k-tiles (one PSUM-wide activation)
  staircase masking applied multiplicatively after exp (fp16)
  PV uses P_T chunks as stationary operand; V carries an extra ones
  column so the softmax denominator accumulates in PSUM alongside O.
"""

import os
import numpy as np

T = 4096
H = 16
HKV = 4
D = 128
BL = 32
NUM_SLOTS = 8192
SCALE = 0.08838834764831845
NCORES = 8
QH = 2                    # query heads per core
QSUP = 512                # queries per supertile (fp32 matmul N max)
NSUP = T // QSUP          # 8
KTILE = 128
NKT = T // KTILE          # 32
KBATCH = 3                # k-tiles per exp batch (PSUM: 2*3 + 2 banks)
DV = D + 1                # V width incl. ones column

_PROG_CACHE = {}
LAST_RESULT = None


def _plan_cache(slot_mapping):
    """Coalesce the cache scatter into contiguous row-range copies.

    Returns segments (dst_start, src_start, n, from_new): from_new rows come
    from the new k/v rows, others pass through the input cache.
    """
    sm = np.asarray(slot_mapping).astype(np.int64)
    src_of = np.full(NUM_SLOTS, -1, np.int64)
    src_of[sm] = np.arange(sm.shape[0])
    segs = []
    r = 0
    while r < NUM_SLOTS:
        if src_of[r] < 0:
            r2 = r
            while r2 < NUM_SLOTS and src_of[r2] < 0:
                r2 += 1
            segs.append((r, r, r2 - r, False))
            r = r2
        else:
            r2 = r
            while r2 + 1 < NUM_SLOTS and src_of[r2 + 1] == src_of[r2] + 1:
                r2 += 1
            segs.append((r, int(src_of[r]), r2 - r + 1, True))
            r = r2 + 1
    return tuple(segs)


def _maybe_patch_ldwopt():
    if not os.environ.get("KNL_LDWOPT"):
        return
    import concourse.bass_utils as bu

    if getattr(bu, "_knl_ldwopt_patched", False):
        return
    orig = bu.run_command

    def patched(cmd, *a, **kw):
        cmd = ["--enable-ldw-opt=true" if c == "--enable-ldw-opt=false" else c
               for c in cmd]
        return orig(cmd, *a, **kw)

    bu.run_command = patched
    bu._knl_ldwopt_patched = True


def _build_program(plan):
    import concourse.mybir as mybir
    from concourse import bacc
    from concourse.tile import TileContext

    _maybe_patch_ldwopt()
    f32 = mybir.dt.float32
    f32r = mybir.dt.float32r
    f16 = mybir.dt.float16
    qk = os.environ.get("KNL_QK", "f16")
    if qk == "bf16":
        fqk = mybir.dt.bfloat16
    elif qk == "f32r":
        fqk = f32r
    else:
        fqk = f16
    EXP = mybir.ActivationFunctionType.Exp

    nc = bacc.Bacc("TRN2", target_bir_lowering=False, debug=False,
                   num_devices=NCORES)

    qT = nc.declare_dram_parameter("qT", [QH, 128, T], fqk, isOutput=False)
    kT = nc.declare_dram_parameter("kT", [128, T], fqk, isOutput=False)
    vp = nc.declare_dram_parameter("vp", [128, NKT * DV], f16, isOutput=False)
    mk = nc.declare_dram_parameter("mk", [128, 128], f16, isOutput=False)
    cin = nc.declare_dram_parameter("cin", [NUM_SLOTS, D], f32, isOutput=False)
    src = nc.declare_dram_parameter("src", [T, D], f32, isOutput=False)
    o_part = nc.declare_dram_parameter("o_part", [QH, T, D], f32, isOutput=True)
    cout = nc.declare_dram_parameter("cout", [NUM_SLOTS, D], f32, isOutput=True)

    with TileContext(nc) as tc:
        with tc.tile_pool(name="const", bufs=1) as cpool, \
             tc.tile_pool(name="work", bufs=4) as wpool, \
             tc.tile_pool(name="stp", bufs=2, space="PSUM") as stpool, \
             tc.tile_pool(name="opsum", bufs=1, space="PSUM") as opool, \
             tc.tile_pool(name="outp", bufs=8) as outpool:

            qT_sb = cpool.tile([128, QH * T], fqk, tag="qT_sb", name="qT_sb")
            kT_sb = cpool.tile([128, T], fqk, tag="kT_sb", name="kT_sb")
            vp_sb = cpool.tile([128, NKT * DV], f16, tag="vp_sb", name="vp_sb")
            mk_sb = cpool.tile([128, 128], f16, tag="mk_sb", name="mk_sb")

            # Pull the ACT exp-table load (~2.7us) into the DMA wait
            # window via a tiny dummy activation on a fresh tile.
            warm = wpool.tile([1, 1], f32, tag="warm", name="warm", bufs=1)
            nc.vector.memset(warm, 0.0)
            nc.scalar.activation(out=warm, in_=warm, func=EXP)

            # Loads, most urgent first (h0/s0 needs kT[0:512], qT h0 s0,
            # vp tiles 0..3, masks).
            nc.sync.dma_start(out=kT_sb[:, 0:128], in_=kT[:, 0:128])
            nc.sync.dma_start(out=qT_sb[:, 0:QSUP], in_=qT[0, :, 0:QSUP])
            nc.sync.dma_start(out=kT_sb[:, 128:512], in_=kT[:, 128:512])
            nc.gpsimd.dma_start(out=vp_sb[:, 0:8 * DV], in_=vp[:, 0:8 * DV])
            nc.gpsimd.dma_start(out=mk_sb[:, :], in_=mk[:, :])
            for j in range(1, 8):
                eng = nc.sync if j % 2 else nc.gpsimd
                eng.dma_start(out=kT_sb[:, j * 512:(j + 1) * 512],
                              in_=kT[:, j * 512:(j + 1) * 512])
            for j in range(1, 4):
                nc.gpsimd.dma_start(out=vp_sb[:, j * 8 * DV:(j + 1) * 8 * DV],
                                    in_=vp[:, j * 8 * DV:(j + 1) * 8 * DV])
            for h in range(QH):
                for s in range(NSUP):
                    if h == 0 and s == 0:
                        continue
                    off = h * T + s * QSUP
                    nc.sync.dma_start(out=qT_sb[:, off:off + QSUP],
                                      in_=qT[h, :, s * QSUP:(s + 1) * QSUP])

            pending = []

            # Diagonal k-tiles only need q >= o*128 (o = in-supertile
            # offset): pack the four restricted-width score tiles into one
            # contiguous PSUM span, ordered so no matmul output crosses a
            # 2KB bank boundary.
            DIAG_SEG = {0: 0, 1: 512, 3: 896, 2: 1024}
            DIAG_W = {0: 512, 1: 384, 2: 256, 3: 128}
            DIAG_TOT = 1280

            def emit_pv(kind, pt, o01, o23, s, extra):
                for ki, c, lcol in _pv_iter(kind, s, extra):
                    ot = o01 if c < 2 else o23
                    col = (c % 2) * DV
                    if kind == "nd":
                        start = (ki == 0 and c % 2 == 0)
                        stop = False
                    else:
                        o = ki - 4 * s
                        start = (s == 0 and o == 0 and c % 2 == 0)
                        stop = (o == 1 and c == 1) or (o == 3 and c == 3)
                        if s == 0 and o == 2 and c == 2:
                            # s=0: o23's first write is in the dg01 pass
                            pass
                    nc.tensor.matmul(
                        ot[:, col:col + DV],
                        lhsT=pt[:, lcol:lcol + 128],
                        rhs=vp_sb[:, ki * DV:(ki + 1) * DV],
                        start=start, stop=stop,
                    )

            def _pv_iter(kind, s, extra):
                if kind == "nd":
                    for j, ki in enumerate(extra):
                        for c in range(4):
                            yield ki, c, j * QSUP + c * 128
                elif kind == "dg01":
                    for o in range(2):
                        for c in range(o, 4):
                            yield 4 * s + o, c, DIAG_SEG[o] + (c - o) * 128
                else:
                    for o in range(2, 4):
                        for c in range(o, 4):
                            yield 4 * s + o, c, DIAG_SEG[o] + (c - o) * 128

            def flush(depth=2):
                while len(pending) > depth:
                    kind, pt, o01, o23, s, h, extra, last = pending.pop(0)
                    _emit_one(kind, pt, o01, o23, s, h, extra, last)

            def _norm_chunk(o01, o23, s, h, c):
                ot = o01 if c < 2 else o23
                col = (c % 2) * DV
                rc = outpool.tile([128, 1], f32, tag="rc", name="rc")
                nc.vector.reciprocal(out=rc, in_=ot[:, col + D:col + DV])
                osb = outpool.tile([128, D], f32, tag="osb", name="osb")
                nc.vector.tensor_scalar_mul(osb, ot[:, col:col + D], rc)
                r0 = s * QSUP + c * 128
                nc.sync.dma_start(out=o_part[h, r0:r0 + 128, :], in_=osb)

            def _emit_one(kind, pt, o01, o23, s, h, extra, last):
                if kind == "nd":
                    emit_pv(kind, pt, o01, o23, s, extra)
                    return
                # diag: o01 is complete after tiles o=0,1 - normalize it
                # while the o23 PVs still run on PE.
                for args in (("dg01",), ("dg23",)):
                    emit_pv(args[0], pt, o01, o23, s, extra)
                    if args[0] == "dg01":
                        _norm_chunk(o01, o23, s, h, 0)
                        _norm_chunk(o01, o23, s, h, 1)
                _norm_chunk(o01, o23, s, h, 2)
                _norm_chunk(o01, o23, s, h, 3)

            for h in range(QH):
                for s in range(NSUP):
                    o01 = opool.tile([128, 2 * DV], f32, tag="o01", name="o01")
                    o23 = opool.tile([128, 2 * DV], f32, tag="o23", name="o23")
                    nnd = 4 * s           # non-diagonal k-tiles
                    qoff = h * T + s * QSUP
                    for b0 in range(0, nnd, KBATCH):
                        batch = list(range(b0, min(b0 + KBATCH, nnd)))
                        nb = len(batch)
                        st = stpool.tile([128, nb * QSUP], f32, tag="st", name="st")
                        for j, ki in enumerate(batch):
                            nc.tensor.matmul(
                                st[:, j * QSUP:(j + 1) * QSUP],
                                lhsT=kT_sb[:, ki * 128:(ki + 1) * 128],
                                rhs=qT_sb[:, qoff:qoff + QSUP],
                                start=True, stop=True,
                            )
                        pt = wpool.tile([128, nb * QSUP], f16, tag="pt", name="pt")
                        nc.scalar.activation(out=pt, in_=st, func=EXP)
                        pending.append(("nd", pt, o01, o23, s, h, batch, False))
                        flush()
                    # diagonal batch: restricted q ranges, one exp
                    st = stpool.tile([128, DIAG_TOT], f32, tag="st", name="st")
                    for o in range(4):
                        ki = 4 * s + o
                        seg, w = DIAG_SEG[o], DIAG_W[o]
                        nc.tensor.matmul(
                            st[:, seg:seg + w],
                            lhsT=kT_sb[:, ki * 128:(ki + 1) * 128],
                            rhs=qT_sb[:, qoff + o * 128:qoff + QSUP],
                            start=True, stop=True,
                        )
                    pt = wpool.tile([128, DIAG_TOT], f16, tag="ptd",
                                    name="ptd", bufs=2)
                    nc.scalar.activation(out=pt, in_=st, func=EXP)
                    for o in range(4):
                        sl = pt[:, DIAG_SEG[o]:DIAG_SEG[o] + 128]
                        nc.vector.tensor_mul(sl, sl, mk_sb[:, 0:128])
                    pending.append(("dg", pt, o01, o23, s, h, None, True))
                    flush()
            flush(depth=0)

            # Cache update: pure DRAM->DRAM copies, chunked <=1024 rows.
            for (dst0, src0, n, from_new) in plan:
                s_t = src if from_new else cin
                for off in range(0, n, 1024):
                    m = min(1024, n - off)
                    nc.sync.dma_start(
                        out=cout[dst0 + off:dst0 + off + m, :],
                        in_=s_t[src0 + off:src0 + off + m, :])

    nc.compile()
    return nc


def _get_program(plan):
    if plan not in _PROG_CACHE:
        _PROG_CACHE[plan] = _build_program(plan)
    return _PROG_CACHE[plan]


def _make_masks():
    tk = np.arange(128)[:, None] // BL          # [128,1] 0..3
    ql = np.arange(128)[None, :] // BL          # [1,128] 0..3
    return (tk <= ql).astype(np.float16)        # [128, 128] local staircase


def kernel(q, k, v, k_cache, v_cache, slot_mapping, block_length):
    global LAST_RESULT
    from concourse.bass_utils import run_bass_kernel_spmd

    q = np.ascontiguousarray(np.asarray(q, dtype=np.float32))
    k = np.ascontiguousarray(np.asarray(k, dtype=np.float32))
    v = np.ascontiguousarray(np.asarray(v, dtype=np.float32))
    k_cache = np.ascontiguousarray(np.asarray(k_cache, dtype=np.float32))
    v_cache = np.ascontiguousarray(np.asarray(v_cache, dtype=np.float32))
    sm = np.asarray(slot_mapping).astype(np.int64)
    assert int(block_length) == BL
    assert q.shape == (T, H * D) and k.shape == (T, HKV * D)

    plan = _plan_cache(sm)
    nc = _get_program(plan)
    qk = os.environ.get("KNL_QK", "f16")
    if qk == "bf16":
        import ml_dtypes
        qk_np = ml_dtypes.bfloat16
    elif qk == "f32r":
        qk_np = np.float32
    else:
        qk_np = np.float16

    qh = q.reshape(T, H, D)
    kh = k.reshape(T, HKV, D)
    vh = v.reshape(T, HKV, D)
    kch = k_cache.reshape(NUM_SLOTS, HKV, D)
    vch = v_cache.reshape(NUM_SLOTS, HKV, D)
    mk = _make_masks()

    in_maps = []
    for c in range(NCORES):
        g = c // 2
        qTc = np.ascontiguousarray(
            (qh[:, 2 * c:2 * c + 2, :] * SCALE).transpose(1, 2, 0)).astype(qk_np)
        kTc = np.ascontiguousarray(kh[:, g, :].T).astype(qk_np)
        vpc = np.ones((T, DV), np.float16)
        vpc[:, :D] = vh[:, g, :].astype(np.float16)
        vpc = np.ascontiguousarray(
            vpc.reshape(NKT, 128, DV).transpose(1, 0, 2).reshape(128, NKT * DV))
        if c % 2 == 0:
            cin = np.ascontiguousarray(kch[:, g, :])
            srcr = np.ascontiguousarray(kh[:, g, :])
        else:
            cin = np.ascontiguousarray(vch[:, g, :])
            srcr = np.ascontiguousarray(vh[:, g, :])
        in_maps.append({"qT": qTc, "kT": kTc, "vp": vpc, "mk": mk,
                        "cin": cin, "src": srcr})

    trace = bool(os.environ.get("KNL_TRACE"))
    if trace:
        try:
            import antenv.axon_hooks  # noqa: F401
        except ImportError:
            trace = False
    res = run_bass_kernel_spmd(nc, in_maps, list(range(NCORES)), trace=trace)
    LAST_RESULT = res

    o = np.empty((T, H, D), np.float32)
    for c in range(NCORES):
        op = res.results[c]["o_part"]          # [QH, T, D]
        o[:, 2 * c, :] = op[0]
        o[:, 2 * c + 1, :] = op[1]
    o = o.reshape(T, H * D)
    kc = np.empty((NUM_SLOTS, HKV * D), np.float32)
    vc = np.empty((NUM_SLOTS, HKV * D), np.float32)
    for c in range(NCORES):
        g = c // 2
        dst = kc if c % 2 == 0 else vc
        dst[:, g * D:(g + 1) * D] = res.results[c]["cout"]
    return o, kc, vc


# revision 15
# speedup vs baseline: 1.0491x; 1.0169x over previous
"""BlockAttention prefill kernel for Trainium2, 8-core tensor-parallel.

Reference op (see problem): scatter K/V rows into paged caches, then
block-causal (staircase, block_length=32) attention over T=4096 tokens,
16 query heads / 4 KV heads (GQA), head_dim=128, fp32.

Sharding: pure tensor parallelism over heads. Core c computes query heads
{2c, 2c+1}, which share KV head c//2. Cache update is split so core 2j
produces the K-cache slice of KV head j and core 2j+1 the V-cache slice.

Per-core kernel layout (one SPMD Bass program, data differs per core):
  S_T[tk, q] = (K_tile)^T-style scores with q streaming (N=512 supertiles)
  exp on ACT in batches of <=3 k-tiles (one PSUM-wide activation)
  staircase masking applied multiplicatively after exp (fp16)
  PV uses P_T chunks as stationary operand; V carries an extra ones
  column so the softmax denominator accumulates in PSUM alongside O.
"""

import os
import numpy as np

T = 4096
H = 16
HKV = 4
D = 128
BL = 32
NUM_SLOTS = 8192
SCALE = 0.08838834764831845
NCORES = 8
QH = 2                    # query heads per core
QSUP = 512                # queries per supertile (fp32 matmul N max)
NSUP = T // QSUP          # 8
KTILE = 128
NKT = T // KTILE          # 32
KBATCH = 3                # k-tiles per exp batch (PSUM: 2*3 + 2 banks)
DV = D + 1                # V width incl. ones column

_PROG_CACHE = {}
LAST_RESULT = None


def _plan_cache(slot_mapping):
    """Coalesce the cache scatter into contiguous row-range copies.

    Returns segments (dst_start, src_start, n, from_new): from_new rows come
    from the new k/v rows, others pass through the input cache.
    """
    sm = np.asarray(slot_mapping).astype(np.int64)
    src_of = np.full(NUM_SLOTS, -1, np.int64)
    src_of[sm] = np.arange(sm.shape[0])
    segs = []
    r = 0
    while r < NUM_SLOTS:
        if src_of[r] < 0:
            r2 = r
            while r2 < NUM_SLOTS and src_of[r2] < 0:
                r2 += 1
            segs.append((r, r, r2 - r, False))
            r = r2
        else:
            r2 = r
            while r2 + 1 < NUM_SLOTS and src_of[r2 + 1] == src_of[r2] + 1:
                r2 += 1
            segs.append((r, int(src_of[r]), r2 - r + 1, True))
            r = r2 + 1
    return tuple(segs)


def _maybe_patch_ldwopt():
    if not os.environ.get("KNL_LDWOPT"):
        return
    import concourse.bass_utils as bu

    if getattr(bu, "_knl_ldwopt_patched", False):
        return
    orig = bu.run_command

    def patched(cmd, *a, **kw):
        cmd = ["--enable-ldw-opt=true" if c == "--enable-ldw-opt=false" else c
               for c in cmd]
        return orig(cmd, *a, **kw)

    bu.run_command = patched
    bu._knl_ldwopt_patched = True


def _build_program(plan):
    import concourse.mybir as mybir
    from concourse import bacc
    from concourse.tile import TileContext

    _maybe_patch_ldwopt()
    f32 = mybir.dt.float32
    f32r = mybir.dt.float32r
    f16 = mybir.dt.float16
    qk = os.environ.get("KNL_QK", "f16")
    if qk == "bf16":
        fqk = mybir.dt.bfloat16
    elif qk == "f32r":
        fqk = f32r
    else:
        fqk = f16
    EXP = mybir.ActivationFunctionType.Exp

    nc = bacc.Bacc("TRN2", target_bir_lowering=False, debug=False,
                   num_devices=NCORES)

    qT = nc.declare_dram_parameter("qT", [QH, 128, T], fqk, isOutput=False)
    kT = nc.declare_dram_parameter("kT", [128, T], fqk, isOutput=False)
    vp = nc.declare_dram_parameter("vp", [128, NKT * DV], f16, isOutput=False)
    mk = nc.declare_dram_parameter("mk", [128, 128], f16, isOutput=False)
    cin = nc.declare_dram_parameter("cin", [NUM_SLOTS, D], f32, isOutput=False)
    src = nc.declare_dram_parameter("src", [T, D], f32, isOutput=False)
    o_part = nc.declare_dram_parameter("o_part", [QH, T, D], f32, isOutput=True)
    cout = nc.declare_dram_parameter("cout", [NUM_SLOTS, D], f32, isOutput=True)

    with TileContext(nc) as tc:
        with tc.tile_pool(name="const", bufs=1) as cpool, \
             tc.tile_pool(name="work", bufs=4) as wpool, \
             tc.tile_pool(name="stp", bufs=2, space="PSUM") as stpool, \
             tc.tile_pool(name="opsum", bufs=1, space="PSUM") as opool, \
             tc.tile_pool(name="outp", bufs=8) as outpool:

            qT_sb = cpool.tile([128, QH * T], fqk, tag="qT_sb", name="qT_sb")
            kT_sb = cpool.tile([128, T], fqk, tag="kT_sb", name="kT_sb")
            vp_sb = cpool.tile([128, NKT * DV], f16, tag="vp_sb", name="vp_sb")
            mk_sb = cpool.tile([128, 128], f16, tag="mk_sb", name="mk_sb")

            # Pull the ACT exp-table load (~2.7us) into the DMA wait
            # window via a tiny dummy activation on a fresh tile.
            warm = wpool.tile([1, 1], f32, tag="warm", name="warm", bufs=1)
            nc.vector.memset(warm, 0.0)
            nc.scalar.activation(out=warm, in_=warm, func=EXP)

            # Loads, most urgent first (h0/s0 needs kT[0:512], qT h0 s0,
            # vp tiles 0..3, masks).
            # Interleave loads in exact consumption order: supertile s of
            # h0 needs qT(s) for its first batch but kT chunk j=s only at
            # its diagonal, so qT(s) issues ahead of kT(s) on the sync
            # queue while vp/mk ride the gpsimd queue.
            nc.sync.dma_start(out=kT_sb[:, 0:128], in_=kT[:, 0:128])
            nc.sync.dma_start(out=qT_sb[:, 0:QSUP], in_=qT[0, :, 0:QSUP])
            nc.sync.dma_start(out=kT_sb[:, 128:512], in_=kT[:, 128:512])
            nc.gpsimd.dma_start(out=vp_sb[:, 0:8 * DV], in_=vp[:, 0:8 * DV])
            nc.gpsimd.dma_start(out=mk_sb[:, :], in_=mk[:, :])
            for s in range(1, NSUP):
                nc.sync.dma_start(out=qT_sb[:, s * QSUP:(s + 1) * QSUP],
                                  in_=qT[0, :, s * QSUP:(s + 1) * QSUP])
                nc.sync.dma_start(out=kT_sb[:, s * 512:(s + 1) * 512],
                                  in_=kT[:, s * 512:(s + 1) * 512])
            for j in range(1, 4):
                nc.gpsimd.dma_start(out=vp_sb[:, j * 8 * DV:(j + 1) * 8 * DV],
                                    in_=vp[:, j * 8 * DV:(j + 1) * 8 * DV])
            for s in range(NSUP):
                off = T + s * QSUP
                nc.gpsimd.dma_start(out=qT_sb[:, off:off + QSUP],
                                    in_=qT[1, :, s * QSUP:(s + 1) * QSUP])

            pending = []

            # Diagonal k-tiles only need q >= o*128 (o = in-supertile
            # offset): pack the four restricted-width score tiles into one
            # contiguous PSUM span, ordered so no matmul output crosses a
            # 2KB bank boundary.
            DIAG_SEG = {0: 0, 1: 512, 3: 896, 2: 1024}
            DIAG_W = {0: 512, 1: 384, 2: 256, 3: 128}
            DIAG_TOT = 1280

            def emit_pv(kind, pt, o01, o23, s, extra):
                for ki, c, lcol in _pv_iter(kind, s, extra):
                    ot = o01 if c < 2 else o23
                    col = (c % 2) * DV
                    if kind == "nd":
                        start = (ki == 0 and c % 2 == 0)
                        stop = False
                    else:
                        o = ki - 4 * s
                        start = (s == 0 and o == 0 and c % 2 == 0)
                        stop = (o == 1 and c == 1) or (o == 3 and c == 3)
                        if s == 0 and o == 2 and c == 2:
                            # s=0: o23's first write is in the dg01 pass
                            pass
                    nc.tensor.matmul(
                        ot[:, col:col + DV],
                        lhsT=pt[:, lcol:lcol + 128],
                        rhs=vp_sb[:, ki * DV:(ki + 1) * DV],
                        start=start, stop=stop,
                    )

            def _pv_iter(kind, s, extra):
                if kind == "nd":
                    for j, ki in enumerate(extra):
                        for c in range(4):
                            yield ki, c, j * QSUP + c * 128
                elif kind == "dg01":
                    for o in range(2):
                        for c in range(o, 4):
                            yield 4 * s + o, c, DIAG_SEG[o] + (c - o) * 128
                else:
                    for o in range(2, 4):
                        for c in range(o, 4):
                            yield 4 * s + o, c, DIAG_SEG[o] + (c - o) * 128

            def flush(depth=2):
                while len(pending) > depth:
                    kind, pt, o01, o23, s, h, extra, last = pending.pop(0)
                    _emit_one(kind, pt, o01, o23, s, h, extra, last)

            def _norm_chunk(o01, o23, s, h, c):
                ot = o01 if c < 2 else o23
                col = (c % 2) * DV
                rc = outpool.tile([128, 1], f32, tag="rc", name="rc")
                nc.vector.reciprocal(out=rc, in_=ot[:, col + D:col + DV])
                osb = outpool.tile([128, D], f32, tag="osb", name="osb")
                nc.vector.tensor_scalar_mul(osb, ot[:, col:col + D], rc)
                r0 = s * QSUP + c * 128
                nc.sync.dma_start(out=o_part[h, r0:r0 + 128, :], in_=osb)

            def _emit_one(kind, pt, o01, o23, s, h, extra, last):
                if kind == "nd":
                    emit_pv(kind, pt, o01, o23, s, extra)
                    return
                # diag: o01 is complete after tiles o=0,1 - normalize it
                # while the o23 PVs still run on PE.
                for args in (("dg01",), ("dg23",)):
                    emit_pv(args[0], pt, o01, o23, s, extra)
                    if args[0] == "dg01":
                        _norm_chunk(o01, o23, s, h, 0)
                        _norm_chunk(o01, o23, s, h, 1)
                _norm_chunk(o01, o23, s, h, 2)
                _norm_chunk(o01, o23, s, h, 3)

            for h in range(QH):
                for s in range(NSUP):
                    o01 = opool.tile([128, 2 * DV], f32, tag="o01", name="o01")
                    o23 = opool.tile([128, 2 * DV], f32, tag="o23", name="o23")
                    nnd = 4 * s           # non-diagonal k-tiles
                    qoff = h * T + s * QSUP
                    for b0 in range(0, nnd, KBATCH):
                        batch = list(range(b0, min(b0 + KBATCH, nnd)))
                        nb = len(batch)
                        st = stpool.tile([128, nb * QSUP], f32, tag="st", name="st")
                        for j, ki in enumerate(batch):
                            nc.tensor.matmul(
                                st[:, j * QSUP:(j + 1) * QSUP],
                                lhsT=kT_sb[:, ki * 128:(ki + 1) * 128],
                                rhs=qT_sb[:, qoff:qoff + QSUP],
                                start=True, stop=True,
                            )
                        pt = wpool.tile([128, nb * QSUP], f16, tag="pt", name="pt")
                        nc.scalar.activation(out=pt, in_=st, func=EXP)
                        pending.append(("nd", pt, o01, o23, s, h, batch, False))
                        flush()
                    # diagonal batch: restricted q ranges, one exp
                    st = stpool.tile([128, DIAG_TOT], f32, tag="st", name="st")
                    for o in range(4):
                        ki = 4 * s + o
                        seg, w = DIAG_SEG[o], DIAG_W[o]
                        nc.tensor.matmul(
                            st[:, seg:seg + w],
                            lhsT=kT_sb[:, ki * 128:(ki + 1) * 128],
                            rhs=qT_sb[:, qoff + o * 128:qoff + QSUP],
                            start=True, stop=True,
                        )
                    pt = wpool.tile([128, DIAG_TOT], f16, tag="ptd",
                                    name="ptd", bufs=2)
                    nc.scalar.activation(out=pt, in_=st, func=EXP)
                    for o in range(4):
                        sl = pt[:, DIAG_SEG[o]:DIAG_SEG[o] + 128]
                        nc.vector.tensor_mul(sl, sl, mk_sb[:, 0:128])
                    pending.append(("dg", pt, o01, o23, s, h, None, True))
                    flush()
            flush(depth=0)

            # Cache update: pure DRAM->DRAM copies, chunked <=1024 rows.
            for (dst0, src0, n, from_new) in plan:
                s_t = src if from_new else cin
                for off in range(0, n, 1024):
                    m = min(1024, n - off)
                    nc.sync.dma_start(
                        out=cout[dst0 + off:dst0 + off + m, :],
                        in_=s_t[src0 + off:src0 + off + m, :])

    nc.compile()
    return nc


def _get_program(plan):
    if plan not in _PROG_CACHE:
        _PROG_CACHE[plan] = _build_program(plan)
    return _PROG_CACHE[plan]


def _make_masks():
    tk = np.arange(128)[:, None] // BL          # [128,1] 0..3
    ql = np.arange(128)[None, :] // BL          # [1,128] 0..3
    return (tk <= ql).astype(np.float16)        # [128, 128] local staircase


def kernel(q, k, v, k_cache, v_cache, slot_mapping, block_length):
    global LAST_RESULT
    from concourse.bass_utils import run_bass_kernel_spmd

    q = np.ascontiguousarray(np.asarray(q, dtype=np.float32))
    k = np.ascontiguousarray(np.asarray(k, dtype=np.float32))
    v = np.ascontiguousarray(np.asarray(v, dtype=np.float32))
    k_cache = np.ascontiguousarray(np.asarray(k_cache, dtype=np.float32))
    v_cache = np.ascontiguousarray(np.asarray(v_cache, dtype=np.float32))
    sm = np.asarray(slot_mapping).astype(np.int64)
    assert int(block_length) == BL
    assert q.shape == (T, H * D) and k.shape == (T, HKV * D)

    plan = _plan_cache(sm)
    nc = _get_program(plan)
    qk = os.environ.get("KNL_QK", "f16")
    if qk == "bf16":
        import ml_dtypes
        qk_np = ml_dtypes.bfloat16
    elif qk == "f32r":
        qk_np = np.float32
    else:
        qk_np = np.float16

    qh = q.reshape(T, H, D)
    kh = k.reshape(T, HKV, D)
    vh = v.reshape(T, HKV, D)
    kch = k_cache.reshape(NUM_SLOTS, HKV, D)
    vch = v_cache.reshape(NUM_SLOTS, HKV, D)
    mk = _make_masks()

    in_maps = []
    for c in range(NCORES):
        g = c // 2
        qTc = np.ascontiguousarray(
            (qh[:, 2 * c:2 * c + 2, :] * SCALE).transpose(1, 2, 0)).astype(qk_np)
        kTc = np.ascontiguousarray(kh[:, g, :].T).astype(qk_np)
        vpc = np.ones((T, DV), np.float16)
        vpc[:, :D] = vh[:, g, :].astype(np.float16)
        vpc = np.ascontiguousarray(
            vpc.reshape(NKT, 128, DV).transpose(1, 0, 2).reshape(128, NKT * DV))
        if c % 2 == 0:
            cin = np.ascontiguousarray(kch[:, g, :])
            srcr = np.ascontiguousarray(kh[:, g, :])
        else:
            cin = np.ascontiguousarray(vch[:, g, :])
            srcr = np.ascontiguousarray(vh[:, g, :])
        in_maps.append({"qT": qTc, "kT": kTc, "vp": vpc, "mk": mk,
                        "cin": cin, "src": srcr})

    trace = bool(os.environ.get("KNL_TRACE"))
    if trace:
        try:
            import antenv.axon_hooks  # noqa: F401
        except ImportError:
            trace = False
    res = run_bass_kernel_spmd(nc, in_maps, list(range(NCORES)), trace=trace)
    LAST_RESULT = res

    o = np.empty((T, H, D), np.float32)
    for c in range(NCORES):
        op = res.results[c]["o_part"]          # [QH, T, D]
        o[:, 2 * c, :] = op[0]
        o[:, 2 * c + 1, :] = op[1]
    o = o.reshape(T, H * D)
    kc = np.empty((NUM_SLOTS, HKV * D), np.float32)
    vc = np.empty((NUM_SLOTS, HKV * D), np.float32)
    for c in range(NCORES):
        g = c // 2
        dst = kc if c % 2 == 0 else vc
        dst[:, g * D:(g + 1) * D] = res.results[c]["cout"]
    return o, kc, vc


# revision 16
# speedup vs baseline: 1.0594x; 1.0098x over previous
"""BlockAttention prefill kernel for Trainium2, 8-core tensor-parallel.

Reference op (see problem): scatter K/V rows into paged caches, then
block-causal (staircase, block_length=32) attention over T=4096 tokens,
16 query heads / 4 KV heads (GQA), head_dim=128, fp32.

Sharding: pure tensor parallelism over heads. Core c computes query heads
{2c, 2c+1}, which share KV head c//2. Cache update is split so core 2j
produces the K-cache slice of KV head j and core 2j+1 the V-cache slice.

Per-core kernel layout (one SPMD Bass program, data differs per core):
  S_T[tk, q] = (K_tile)^T-style scores with q streaming (N=512 supertiles)
  exp on ACT in batches of <=3 k-tiles (one PSUM-wide activation)
  staircase masking applied multiplicatively after exp (fp16)
  PV uses P_T chunks as stationary operand; V carries an extra ones
  column so the softmax denominator accumulates in PSUM alongside O.
"""

import os
import numpy as np

T = 4096
H = 16
HKV = 4
D = 128
BL = 32
NUM_SLOTS = 8192
SCALE = 0.08838834764831845
NCORES = 8
QH = 2                    # query heads per core
QSUP = 512                # queries per supertile (fp32 matmul N max)
NSUP = T // QSUP          # 8
KTILE = 128
NKT = T // KTILE          # 32
KBATCH = 3                # k-tiles per exp batch (PSUM: 2*3 + 2 banks)
DV = D + 1                # V width incl. ones column

_PROG_CACHE = {}
LAST_RESULT = None


def _plan_cache(slot_mapping):
    """Coalesce the cache scatter into contiguous row-range copies.

    Returns segments (dst_start, src_start, n, from_new): from_new rows come
    from the new k/v rows, others pass through the input cache.
    """
    sm = np.asarray(slot_mapping).astype(np.int64)
    src_of = np.full(NUM_SLOTS, -1, np.int64)
    src_of[sm] = np.arange(sm.shape[0])
    segs = []
    r = 0
    while r < NUM_SLOTS:
        if src_of[r] < 0:
            r2 = r
            while r2 < NUM_SLOTS and src_of[r2] < 0:
                r2 += 1
            segs.append((r, r, r2 - r, False))
            r = r2
        else:
            r2 = r
            while r2 + 1 < NUM_SLOTS and src_of[r2 + 1] == src_of[r2] + 1:
                r2 += 1
            segs.append((r, int(src_of[r]), r2 - r + 1, True))
            r = r2 + 1
    return tuple(segs)


def _maybe_patch_ldwopt():
    if not os.environ.get("KNL_LDWOPT"):
        return
    import concourse.bass_utils as bu

    if getattr(bu, "_knl_ldwopt_patched", False):
        return
    orig = bu.run_command

    def patched(cmd, *a, **kw):
        cmd = ["--enable-ldw-opt=true" if c == "--enable-ldw-opt=false" else c
               for c in cmd]
        return orig(cmd, *a, **kw)

    bu.run_command = patched
    bu._knl_ldwopt_patched = True


def _build_program(plan):
    import concourse.mybir as mybir
    from concourse import bacc
    from concourse.tile import TileContext

    _maybe_patch_ldwopt()
    f32 = mybir.dt.float32
    f32r = mybir.dt.float32r
    f16 = mybir.dt.float16
    qk = os.environ.get("KNL_QK", "f16")
    if qk == "bf16":
        fqk = mybir.dt.bfloat16
    elif qk == "f32r":
        fqk = f32r
    else:
        fqk = f16
    EXP = mybir.ActivationFunctionType.Exp

    nc = bacc.Bacc("TRN2", target_bir_lowering=False, debug=False,
                   num_devices=NCORES)

    qT = nc.declare_dram_parameter("qT", [QH, 128, T], fqk, isOutput=False)
    kT = nc.declare_dram_parameter("kT", [128, T], fqk, isOutput=False)
    vp = nc.declare_dram_parameter("vp", [128, NKT * DV], f16, isOutput=False)
    mk = nc.declare_dram_parameter("mk", [128, 128], f16, isOutput=False)
    cin = nc.declare_dram_parameter("cin", [NUM_SLOTS, D], f32, isOutput=False)
    src = nc.declare_dram_parameter("src", [T, D], f32, isOutput=False)
    o_part = nc.declare_dram_parameter("o_part", [QH, T, D], f32, isOutput=True)
    cout = nc.declare_dram_parameter("cout", [NUM_SLOTS, D], f32, isOutput=True)

    with TileContext(nc) as tc:
        with tc.tile_pool(name="const", bufs=1) as cpool, \
             tc.tile_pool(name="work", bufs=5) as wpool, \
             tc.tile_pool(name="stp", bufs=2, space="PSUM") as stpool, \
             tc.tile_pool(name="opsum", bufs=1, space="PSUM") as opool, \
             tc.tile_pool(name="outp", bufs=8) as outpool:

            qT_sb = cpool.tile([128, QH * T], fqk, tag="qT_sb", name="qT_sb")
            kT_sb = cpool.tile([128, T], fqk, tag="kT_sb", name="kT_sb")
            vp_sb = cpool.tile([128, NKT * DV], f16, tag="vp_sb", name="vp_sb")
            mk_sb = cpool.tile([128, 128], f16, tag="mk_sb", name="mk_sb")

            # Pull the ACT exp-table load (~2.7us) into the DMA wait
            # window via a tiny dummy activation on a fresh tile.
            warm = wpool.tile([1, 1], f32, tag="warm", name="warm", bufs=1)
            nc.vector.memset(warm, 0.0)
            nc.scalar.activation(out=warm, in_=warm, func=EXP)

            # Loads, most urgent first (h0/s0 needs kT[0:512], qT h0 s0,
            # vp tiles 0..3, masks).
            # Interleave loads in exact consumption order: supertile s of
            # h0 needs qT(s) for its first batch but kT chunk j=s only at
            # its diagonal, so qT(s) issues ahead of kT(s) on the sync
            # queue while vp/mk ride the gpsimd queue.
            nc.sync.dma_start(out=kT_sb[:, 0:128], in_=kT[:, 0:128])
            nc.sync.dma_start(out=qT_sb[:, 0:QSUP], in_=qT[0, :, 0:QSUP])
            nc.sync.dma_start(out=kT_sb[:, 128:512], in_=kT[:, 128:512])
            nc.gpsimd.dma_start(out=vp_sb[:, 0:8 * DV], in_=vp[:, 0:8 * DV])
            nc.gpsimd.dma_start(out=mk_sb[:, :], in_=mk[:, :])
            for s in range(1, NSUP):
                nc.sync.dma_start(out=qT_sb[:, s * QSUP:(s + 1) * QSUP],
                                  in_=qT[0, :, s * QSUP:(s + 1) * QSUP])
                nc.sync.dma_start(out=kT_sb[:, s * 512:(s + 1) * 512],
                                  in_=kT[:, s * 512:(s + 1) * 512])
            for j in range(1, 4):
                nc.gpsimd.dma_start(out=vp_sb[:, j * 8 * DV:(j + 1) * 8 * DV],
                                    in_=vp[:, j * 8 * DV:(j + 1) * 8 * DV])
            for s in range(NSUP):
                off = T + s * QSUP
                nc.gpsimd.dma_start(out=qT_sb[:, off:off + QSUP],
                                    in_=qT[1, :, s * QSUP:(s + 1) * QSUP])

            pending = []

            # Diagonal k-tiles only need q >= o*128 (o = in-supertile
            # offset): pack the four restricted-width score tiles into one
            # contiguous PSUM span, ordered so no matmul output crosses a
            # 2KB bank boundary.
            DIAG_SEG = {0: 0, 1: 512, 3: 896, 2: 1024}
            DIAG_W = {0: 512, 1: 384, 2: 256, 3: 128}
            DIAG_TOT = 1280

            def emit_pv(kind, pt, o01, o23, s, extra):
                for ki, c, lcol in _pv_iter(kind, s, extra):
                    ot = o01 if c < 2 else o23
                    col = (c % 2) * DV
                    if kind == "nd":
                        start = (ki == 0 and c % 2 == 0)
                        stop = False
                    else:
                        o = ki - 4 * s
                        start = (s == 0 and o == 0 and c % 2 == 0)
                        stop = (o == 1 and c == 1) or (o == 3 and c == 3)
                        if s == 0 and o == 2 and c == 2:
                            # s=0: o23's first write is in the dg01 pass
                            pass
                    nc.tensor.matmul(
                        ot[:, col:col + DV],
                        lhsT=pt[:, lcol:lcol + 128],
                        rhs=vp_sb[:, ki * DV:(ki + 1) * DV],
                        start=start, stop=stop,
                    )

            def _pv_iter(kind, s, extra):
                if kind == "nd":
                    for j, ki in enumerate(extra):
                        for c in range(4):
                            yield ki, c, j * QSUP + c * 128
                elif kind == "dg01":
                    for o in range(2):
                        for c in range(o, 4):
                            yield 4 * s + o, c, DIAG_SEG[o] + (c - o) * 128
                else:
                    for o in range(2, 4):
                        for c in range(o, 4):
                            yield 4 * s + o, c, DIAG_SEG[o] + (c - o) * 128

            def flush(depth=3):
                while len(pending) > depth:
                    kind, pt, o01, o23, s, h, extra, last = pending.pop(0)
                    _emit_one(kind, pt, o01, o23, s, h, extra, last)

            def _norm_chunk(o01, o23, s, h, c):
                ot = o01 if c < 2 else o23
                col = (c % 2) * DV
                rc = outpool.tile([128, 1], f32, tag="rc", name="rc")
                nc.vector.reciprocal(out=rc, in_=ot[:, col + D:col + DV])
                osb = outpool.tile([128, D], f32, tag="osb", name="osb")
                nc.vector.tensor_scalar_mul(osb, ot[:, col:col + D], rc)
                r0 = s * QSUP + c * 128
                nc.sync.dma_start(out=o_part[h, r0:r0 + 128, :], in_=osb)

            def _emit_one(kind, pt, o01, o23, s, h, extra, last):
                if kind == "nd":
                    emit_pv(kind, pt, o01, o23, s, extra)
                    return
                # diag: o01 is complete after tiles o=0,1 - normalize it
                # while the o23 PVs still run on PE.
                for args in (("dg01",), ("dg23",)):
                    emit_pv(args[0], pt, o01, o23, s, extra)
                    if args[0] == "dg01":
                        _norm_chunk(o01, o23, s, h, 0)
                        _norm_chunk(o01, o23, s, h, 1)
                _norm_chunk(o01, o23, s, h, 2)
                _norm_chunk(o01, o23, s, h, 3)

            for h in range(QH):
                for s in range(NSUP):
                    o01 = opool.tile([128, 2 * DV], f32, tag="o01", name="o01")
                    o23 = opool.tile([128, 2 * DV], f32, tag="o23", name="o23")
                    nnd = 4 * s           # non-diagonal k-tiles
                    qoff = h * T + s * QSUP
                    for b0 in range(0, nnd, KBATCH):
                        batch = list(range(b0, min(b0 + KBATCH, nnd)))
                        nb = len(batch)
                        st = stpool.tile([128, nb * QSUP], f32, tag="st", name="st")
                        for j, ki in enumerate(batch):
                            nc.tensor.matmul(
                                st[:, j * QSUP:(j + 1) * QSUP],
                                lhsT=kT_sb[:, ki * 128:(ki + 1) * 128],
                                rhs=qT_sb[:, qoff:qoff + QSUP],
                                start=True, stop=True,
                            )
                        pt = wpool.tile([128, nb * QSUP], f16, tag="pt", name="pt")
                        nc.scalar.activation(out=pt, in_=st, func=EXP)
                        pending.append(("nd", pt, o01, o23, s, h, batch, False))
                        flush()
                    # diagonal batch: restricted q ranges, one exp
                    st = stpool.tile([128, DIAG_TOT], f32, tag="st", name="st")
                    for o in range(4):
                        ki = 4 * s + o
                        seg, w = DIAG_SEG[o], DIAG_W[o]
                        nc.tensor.matmul(
                            st[:, seg:seg + w],
                            lhsT=kT_sb[:, ki * 128:(ki + 1) * 128],
                            rhs=qT_sb[:, qoff + o * 128:qoff + QSUP],
                            start=True, stop=True,
                        )
                    pt = wpool.tile([128, DIAG_TOT], f16, tag="ptd",
                                    name="ptd", bufs=3)
                    nc.scalar.activation(out=pt, in_=st, func=EXP)
                    for o in range(4):
                        sl = pt[:, DIAG_SEG[o]:DIAG_SEG[o] + 128]
                        nc.vector.tensor_mul(sl, sl, mk_sb[:, 0:128])
                    pending.append(("dg", pt, o01, o23, s, h, None, True))
                    flush()
            flush(depth=0)

            # Cache update: pure DRAM->DRAM copies, chunked <=1024 rows.
            for (dst0, src0, n, from_new) in plan:
                s_t = src if from_new else cin
                for off in range(0, n, 1024):
                    m = min(1024, n - off)
                    nc.sync.dma_start(
                        out=cout[dst0 + off:dst0 + off + m, :],
                        in_=s_t[src0 + off:src0 + off + m, :])

    nc.compile()
    return nc


def _get_program(plan):
    if plan not in _PROG_CACHE:
        _PROG_CACHE[plan] = _build_program(plan)
    return _PROG_CACHE[plan]


def _make_masks():
    tk = np.arange(128)[:, None] // BL          # [128,1] 0..3
    ql = np.arange(128)[None, :] // BL          # [1,128] 0..3
    return (tk <= ql).astype(np.float16)        # [128, 128] local staircase


def kernel(q, k, v, k_cache, v_cache, slot_mapping, block_length):
    global LAST_RESULT
    from concourse.bass_utils import run_bass_kernel_spmd

    q = np.ascontiguousarray(np.asarray(q, dtype=np.float32))
    k = np.ascontiguousarray(np.asarray(k, dtype=np.float32))
    v = np.ascontiguousarray(np.asarray(v, dtype=np.float32))
    k_cache = np.ascontiguousarray(np.asarray(k_cache, dtype=np.float32))
    v_cache = np.ascontiguousarray(np.asarray(v_cache, dtype=np.float32))
    sm = np.asarray(slot_mapping).astype(np.int64)
    assert int(block_length) == BL
    assert q.shape == (T, H * D) and k.shape == (T, HKV * D)

    plan = _plan_cache(sm)
    nc = _get_program(plan)
    qk = os.environ.get("KNL_QK", "f16")
    if qk == "bf16":
        import ml_dtypes
        qk_np = ml_dtypes.bfloat16
    elif qk == "f32r":
        qk_np = np.float32
    else:
        qk_np = np.float16

    qh = q.reshape(T, H, D)
    kh = k.reshape(T, HKV, D)
    vh = v.reshape(T, HKV, D)
    kch = k_cache.reshape(NUM_SLOTS, HKV, D)
    vch = v_cache.reshape(NUM_SLOTS, HKV, D)
    mk = _make_masks()

    in_maps = []
    for c in range(NCORES):
        g = c // 2
        qTc = np.ascontiguousarray(
            (qh[:, 2 * c:2 * c + 2, :] * SCALE).transpose(1, 2, 0)).astype(qk_np)
        kTc = np.ascontiguousarray(kh[:, g, :].T).astype(qk_np)
        vpc = np.ones((T, DV), np.float16)
        vpc[:, :D] = vh[:, g, :].astype(np.float16)
        vpc = np.ascontiguousarray(
            vpc.reshape(NKT, 128, DV).transpose(1, 0, 2).reshape(128, NKT * DV))
        if c % 2 == 0:
            cin = np.ascontiguousarray(kch[:, g, :])
            srcr = np.ascontiguousarray(kh[:, g, :])
        else:
            cin = np.ascontiguousarray(vch[:, g, :])
            srcr = np.ascontiguousarray(vh[:, g, :])
        in_maps.append({"qT": qTc, "kT": kTc, "vp": vpc, "mk": mk,
                        "cin": cin, "src": srcr})

    trace = bool(os.environ.get("KNL_TRACE"))
    if trace:
        try:
            import antenv.axon_hooks  # noqa: F401
        except ImportError:
            trace = False
    res = run_bass_kernel_spmd(nc, in_maps, list(range(NCORES)), trace=trace)
    LAST_RESULT = res

    o = np.empty((T, H, D), np.float32)
    for c in range(NCORES):
        op = res.results[c]["o_part"]          # [QH, T, D]
        o[:, 2 * c, :] = op[0]
        o[:, 2 * c + 1, :] = op[1]
    o = o.reshape(T, H * D)
    kc = np.empty((NUM_SLOTS, HKV * D), np.float32)
    vc = np.empty((NUM_SLOTS, HKV * D), np.float32)
    for c in range(NCORES):
        g = c // 2
        dst = kc if c % 2 == 0 else vc
        dst[:, g * D:(g + 1) * D] = res.results[c]["cout"]
    return o, kc, vc
